# revision 43
# baseline (speedup 1.0000x reference)
"""CoreAttention on 8 Trainium2 cores.

Sharding: 32 (batch, head) pairs -> 4 heads per core (cores 0-3: batch 0,
cores 4-7: batch 1). Per core, per head: scores^T = K Q^T in [t, s]
orientation with bf16 operands (fp32 PSUM accumulate), exp on ACT writing
bf16, mask applied as bf16 multiplies on DVE (2x perf mode, 4-chunk-wide
FD=2048 ops). Column sums split to balance PE vs DVE: chunks 0..11
pair-tree-added on DVE into a [128,2,SBLK] accumulator (wide FD=1024 bf16
adds), chunks 12..15 plus the two accumulator rows contracted on the PE.
The sums contraction uses a full ones [128,128] stationary, so the column
sums land already replicated across all 128 partitions (same N=512
streaming cost as an M=1 join) — reciprocal and the normalize multiply
then run directly on the DVE with no partition_broadcast and no GPSIMD.
P@V as bf16 matmuls accumulating in PSUM. Double-buffered sums/ctx banks
so the PE never waits on the DVE's reads.

Hard-won scheduling facts baked in here:
- GPSIMD shares an SBUF port with the DVE: ANY heavy gpsimd elementwise
  work slows every DVE op ~20% and its strict FIFO head-of-line blocks
  cross-engine chains. Keep gpsimd idle.
- Each dma_start costs ~0.6us of serialized SP sequencer time (DIRECT2D),
  and consumers wait on whole-DMA semaphores: use FEW transfers, sized to
  need-granularity, issued in consumption order. The keep mask is
  host-swizzled to [p, sb, c, s'] so one s-block is one DMA with
  16KB-contiguous runs per partition.
- Software pipeline DEPTH=2 (deeper measured slower); the final block
  drains with the back phase split into s-halves to shorten the tail.
Host side only slices/transposes/casts inputs (layout prep).
"""
import sys, math
import numpy as np

sys.path.insert(0, "/opt/trn_rl_repo")

SQ, B, NH, HN = 2048, 2, 16, 128
NCORES = 8
HPC = 4                      # heads per core
TC = SQ // 128               # 16 t-chunks
SBLK = 512                   # s-block width
NSB = SQ // SBLK             # 4 s-blocks
SCALE = 1.0 / math.sqrt(128.0)   # COEFF / NORM_FACTOR = 1/sqrt(hn)
KDVE = 12                        # t-chunks whose column-sums go via DVE adds
DEPTH = 2                        # software pipeline depth (F blocks ahead of B)

_CACHE = {}


def _build(repeat=1):
    import concourse.bacc as bacc
    import concourse.tile as tile
    from concourse import mybir

    F32, BF16, U32 = mybir.dt.float32, mybir.dt.bfloat16, mybir.dt.uint32
    EXP = mybir.ActivationFunctionType.Exp
    AND = mybir.AluOpType.logical_and

    nc = bacc.Bacc(None, target_bir_lowering=False)
    qT_d = nc.dram_tensor("qT", [HPC, HN, SQ], BF16, kind="ExternalInput")
    kT_d = nc.dram_tensor("kT", [HPC, HN, SQ], BF16, kind="ExternalInput")
    v_d = nc.dram_tensor("v", [HPC, SQ, HN], BF16, kind="ExternalInput")
    # keep swizzled host-side to [p, sb, c, s'] so one s-block is a single
    # DMA with 16KB-contiguous runs per partition (descriptor-efficient)
    keep_d = nc.dram_tensor("keepT", [128, NSB, TC, SBLK], BF16,
                            kind="ExternalInput")
    ctxT_d = nc.dram_tensor("ctxT", [HPC, HN, SQ], BF16, kind="ExternalOutput")

    with tile.TileContext(nc) as tc:
        with (
            tc.tile_pool(name="sbkeep", bufs=1) as sbkeep,
            tc.tile_pool(name="const", bufs=1) as const,
            tc.tile_pool(name="sbqkv", bufs=3) as sbqkv,
            tc.tile_pool(name="sbpt", bufs=DEPTH + 1) as sbpt,
            tc.tile_pool(name="sbacc", bufs=DEPTH + 1) as sbacc,
            tc.tile_pool(name="sbtmp", bufs=2) as sbtmp,
            tc.tile_pool(name="sbe", bufs=4) as sbe,
            tc.tile_pool(name="sbmisc", bufs=3) as sbmisc,
            tc.tile_pool(name="pst", bufs=2, space="PSUM") as pst,
            tc.tile_pool(name="psc", bufs=2, space="PSUM") as psc,
            tc.tile_pool(name="pss", bufs=2, space="PSUM") as pss,
        ):
            keep_t = sbkeep.tile([128, NSB, TC, SBLK], BF16, tag="keep")

            ones_b = const.tile([128, 1], BF16, tag="ob")
            nc.vector.memset(ones_b[:], 1.0)
            # full ones stationary: the sums joins then produce the column
            # sums replicated across all 128 partitions (same N=512
            # streaming cost as an M=1 join), so no partition_broadcast
            # is ever needed for the normalize
            ones_m = const.tile([128, 128], BF16, tag="om")
            nc.vector.memset(ones_m[:], 1.0)
            warm_src = const.tile([128, SBLK], BF16, tag="warm")
            nc.vector.memset(warm_src[:], 0.0)
            warm_e = const.tile([128, 16], BF16, tag="warme")

            def emit_front(h, sb, qT_t, kT_t):
                """scores -> exp -> mask for (h, sb); returns (pt, acc).

                Mask is a u32 bitwise AND (keep pattern 0xFFFF/0x0000);
                the q=3 group's AND runs on GPSIMD (its chunks feed only
                PE matmuls, so no DVE chain depends on it). Chunks
                0..KDVE-1 are pair-tree-reduced on the DVE into
                acc[128, 2, SBLK]; the PE later contracts chunks
                KDVE..15 plus the two acc rows.
                """
                s0 = sb * SBLK
                pt = sbpt.tile([128, TC, SBLK], BF16, tag="pt")
                acc = sbacc.tile([128, 2, SBLK], BF16, tag="acc")
                tmp = sbtmp.tile([128, 2, SBLK], BF16, tag="tmp")
                # NOTE: gpsimd must not take any of this work — it shares
                # an SBUF port with the DVE, and heavy gpsimd elementwise
                # traffic slows every DVE op by ~20% (measured)
                for h8 in range(2):
                    e16 = sbe.tile([128, 8, SBLK], BF16, tag="e")
                    for g in range(4):
                        st = pst.tile([128, 2, SBLK], F32, tag="st")
                        for j in range(2):
                            ti = 8 * h8 + 2 * g + j
                            nc.tensor.matmul(
                                st[:, j, :],
                                kT_t[:, 128 * ti:128 * (ti + 1)],
                                qT_t[:, s0:s0 + SBLK],
                                start=True, stop=True)
                        nc.scalar.activation(
                            e16[:, 2 * g:2 * g + 2, :], st[:], EXP,
                            scale=SCALE)
                    c0 = 8 * h8
                    nc.vector.tensor_mul(
                        pt[:, c0:c0 + 8, :], e16[:],
                        keep_t[:, sb, c0:c0 + 8, :])
                    if h8 == 0:
                        nc.vector.tensor_add(acc[:], pt[:, 0:2, :],
                                             pt[:, 2:4, :])
                        nc.vector.tensor_add(tmp[:], pt[:, 4:6, :],
                                             pt[:, 6:8, :])
                        nc.vector.tensor_add(acc[:], acc[:], tmp[:])
                    else:
                        nc.vector.tensor_add(tmp[:], pt[:, 8:10, :],
                                             pt[:, 10:12, :])
                        nc.vector.tensor_add(acc[:], acc[:], tmp[:])
                return pt, acc

            def emit_back(h, sb, pt, acc, v_t, nsplit=1, shalves=1):
                """sums -> PV -> normalize -> store for (h, sb).

                shalves > 1 splits the whole back phase into s-column
                halves so the drain chain after the final PV matmul is
                half as long (used for the last pipeline block only).
                """
                s0 = sb * SBLK
                sw = SBLK // shalves
                for so in range(0, SBLK, sw):
                    sums_p = pss.tile([128, SBLK], F32, tag="sums")
                    for ti in range(KDVE, TC):
                        nc.tensor.matmul(sums_p[:, :sw], ones_m[:],
                                         pt[:, ti, so:so + sw],
                                         start=(ti == KDVE), stop=False)
                    nc.tensor.matmul(sums_p[:, :sw], ones_m[:],
                                     acc[:, 0, so:so + sw],
                                     start=False, stop=False)
                    nc.tensor.matmul(sums_p[:, :sw], ones_m[:],
                                     acc[:, 1, so:so + sw],
                                     start=False, stop=True)
                    ctx_p = psc.tile([128, SBLK], F32, tag="ctx")
                    for ti in range(TC):
                        nc.tensor.matmul(ctx_p[:, :sw], v_t[:, ti, :],
                                         pt[:, ti, so:so + sw],
                                         start=(ti == 0), stop=(ti == TC - 1))
                    w = sw // nsplit
                    for o in range(0, sw, w):
                        recip = sbmisc.tile([128, w], F32, tag="recip")
                        nc.vector.reciprocal_approx_fast(
                            recip[:], sums_p[:, o:o + w])
                        ctx_s = sbmisc.tile([128, w], BF16, tag="ctxs")
                        nc.vector.tensor_mul(ctx_s[:], ctx_p[:, o:o + w],
                                             recip[:])
                        nc.sync.dma_start(
                            out=ctxT_d[h, :, s0 + so + o:s0 + so + o + w],
                            in_=ctx_s[:])

            def body(_iv=None):
                # warm the PE clock (HAM) and the ACT exp table with dummy
                # ops that only depend on the memset, while the first DMAs
                # land
                warm_p = pss.tile([1, SBLK], F32, tag="sums")
                for _ in range(6):
                    nc.tensor.matmul(warm_p[:], ones_b[:], warm_src[:],
                                     start=True, stop=True)
                nc.scalar.activation(warm_e[:], warm_src[:, 0:16], EXP,
                                     scale=SCALE)

                pendings = []   # [(h, sb, pt, acc, v_t), ...]
                qkv = {}
                for h in range(HPC):
                    qT_t = sbqkv.tile([128, SQ], BF16, tag="qT")
                    kT_t = sbqkv.tile([128, SQ], BF16, tag="kT")
                    v_t = sbqkv.tile([128, TC, HN], BF16, tag="v")
                    v_r = v_d[h].rearrange("(c p) d -> p c d", p=128)
                    if h == 0:
                        # need-ordered loads, trigger-frugal: each
                        # dma_start costs ~0.6us of SP sequencer time
                        # (DIRECT2D), and consumers wait on whole-DMA
                        # semaphores — so the first-needed operands go
                        # as small pieces and the bulk as few large
                        # descriptor-efficient transfers.
                        # qT s-block 0 + first kT chunks gate compute
                        # start; issue them from the SECOND HWDGE engine
                        # (ACT, idle until its first exp at ~12us) so
                        # their DIRECT2D triggers fire in parallel with
                        # the SP's stream below
                        for c in range(2):
                            nc.scalar.dma_start(
                                out=qT_t[:, 256 * c:256 * (c + 1)],
                                in_=qT_d[h][:, 256 * c:256 * (c + 1)])
                        # kT as 16 x 128-col pieces, alternating between
                        # the two HWDGE trigger engines for ring
                        # parallelism without delaying SP's keep triggers
                        for c in range(16):
                            eng = nc.scalar if c % 2 else nc.sync
                            eng.dma_start(
                                out=kT_t[:, 128 * c:128 * (c + 1)],
                                in_=kT_d[h][:, 128 * c:128 * (c + 1)])
                        for qtr in range(4):
                            nc.sync.dma_start(
                                out=keep_t[:, 0, 4 * qtr:4 * (qtr + 1), :],
                                in_=keep_d[:, 0, 4 * qtr:4 * (qtr + 1), :])
                        nc.sync.dma_start(
                            out=keep_t[:, 1, 0:8, :],
                            in_=keep_d[:, 1, 0:8, :])
                        nc.sync.dma_start(out=qT_t[:, SBLK:2 * SBLK],
                                          in_=qT_d[h][:, SBLK:2 * SBLK])
                        nc.sync.dma_start(
                            out=keep_t[:, 1, 8:16, :],
                            in_=keep_d[:, 1, 8:16, :])
                        for c in range(2, NSB):
                            nc.sync.dma_start(
                                out=qT_t[:, SBLK * c:SBLK * (c + 1)],
                                in_=qT_d[h][:, SBLK * c:SBLK * (c + 1)])
                        for half in range(2):
                            nc.sync.dma_start(
                                out=v_t[:, 8 * half:8 * (half + 1), :],
                                in_=v_r[:, 8 * half:8 * (half + 1), :])
                        for half in range(2):
                            nc.sync.dma_start(
                                out=keep_t[:, 2, 8 * half:8 * (half + 1), :],
                                in_=keep_d[:, 2, 8 * half:8 * (half + 1), :])
                        nc.sync.dma_start(out=keep_t[:, 3], in_=keep_d[:, 3])
                    else:
                        for half in range(2):
                            cols = slice(SQ // 2 * half, SQ // 2 * (half + 1))
                            nc.sync.dma_start(out=qT_t[:, cols],
                                              in_=qT_d[h][:, cols])
                            nc.sync.dma_start(out=kT_t[:, cols],
                                              in_=kT_d[h][:, cols])
                            nc.sync.dma_start(
                                out=v_t[:, 8 * half:8 * (half + 1), :],
                                in_=v_r[:, 8 * half:8 * (half + 1), :])
                    qkv[h] = (qT_t, kT_t, v_t)
                    last_head = h == HPC - 1
                    for sb in range(NSB):
                        # shallow out the pipeline over the last two
                        # blocks (one extra back each, instead of a
                        # double-drain burst before the final front) so
                        # the drain tail after the last front is short
                        if last_head and sb >= NSB - 2:
                            while len(pendings) > 1:
                                emit_back(*pendings.pop(0))
                        pt, acc = emit_front(h, sb, qT_t, kT_t)
                        pendings.append((h, sb, pt, acc, v_t))
                        if len(pendings) > DEPTH:
                            emit_back(*pendings.pop(0))
                for i, p in enumerate(pendings):
                    emit_back(*p, shalves=1 + i)

            if repeat == 1:
                body()
            else:
                with tc.For_i(0, repeat, 1):
                    body()
    nc.compile()
    return nc


def _get_nc(repeat=1):
    if repeat not in _CACHE:
        _CACHE[repeat] = _build(repeat)
    return _CACHE[repeat]


def _make_in_maps(query_layer, key_layer, value_layer, attention_mask):
    import ml_dtypes
    bf16 = ml_dtypes.bfloat16
    q = np.asarray(query_layer, dtype=np.float32)
    k = np.asarray(key_layer, dtype=np.float32)
    v = np.asarray(value_layer, dtype=np.float32)
    m = np.asarray(attention_mask)
    in_maps = []
    for c in range(NCORES):
        b = c // 4
        h0 = 4 * (c % 4)
        hs = slice(h0, h0 + HPC)
        qT = np.ascontiguousarray(
            q[:, b, hs, :].transpose(1, 2, 0)).astype(bf16)    # [4,hn,sq]
        kT = np.ascontiguousarray(
            k[:, b, hs, :].transpose(1, 2, 0)).astype(bf16)
        vv = np.ascontiguousarray(
            v[:, b, hs, :].transpose(1, 0, 2)).astype(bf16)    # [4,sq,hn]
        # keep swizzled to [p, sb, c, s']: keep2[p, sb, c, s'] =
        # keep[t = c*128 + p, s = sb*512 + s']
        keep_ts = (m[b, 0] == 0).T.astype(bf16)                # [t,s] bf16
        keepT = np.ascontiguousarray(
            keep_ts.reshape(16, 128, 4, 512).transpose(1, 2, 0, 3))
        in_maps.append({"qT": qT, "kT": kT, "v": vv, "keepT": keepT})
    return in_maps


def _run(nc, in_maps):
    from concourse.bass_utils import run_bass_kernel_spmd
    return run_bass_kernel_spmd(nc, in_maps, list(range(NCORES)))


def kernel(query_layer, key_layer, value_layer, attention_mask):
    in_maps = _make_in_maps(query_layer, key_layer, value_layer, attention_mask)
    res = _run(_get_nc(1), in_maps)
    out = np.empty((SQ, B, NH, HN), dtype=np.float32)
    for c in range(NCORES):
        b = c // 4
        h0 = 4 * (c % 4)
        ctxT = np.asarray(res.results[c]["ctxT"], dtype=np.float32)   # [4,hn,sq]
        out[:, b, h0:h0 + HPC, :] = ctxT.transpose(2, 0, 1)
    return out.reshape(SQ, B, NH * HN)


# revision 45
# speedup vs baseline: 1.0433x; 1.0433x over previous
"""CoreAttention on 8 Trainium2 cores.

Sharding: 32 (batch, head) pairs -> 4 heads per core (cores 0-3: batch 0,
cores 4-7: batch 1). Per core, per head: scores^T = K Q^T in [t, s]
orientation with bf16 operands (fp32 PSUM accumulate), exp on ACT writing
bf16, mask applied as bf16 multiplies on DVE (2x perf mode, 4-chunk-wide
FD=2048 ops). Column sums split to balance PE vs DVE: chunks 0..11
pair-tree-added on DVE into a [128,2,SBLK] accumulator (wide FD=1024 bf16
adds), chunks 12..15 plus the two accumulator rows contracted on the PE.
The sums contraction uses a full ones [128,128] stationary, so the column
sums land already replicated across all 128 partitions (same N=512
streaming cost as an M=1 join) — reciprocal and the normalize multiply
then run directly on the DVE with no partition_broadcast and no GPSIMD.
P@V as bf16 matmuls accumulating in PSUM. Double-buffered sums/ctx banks
so the PE never waits on the DVE's reads.

Hard-won scheduling facts baked in here:
- GPSIMD shares an SBUF port with the DVE: ANY heavy gpsimd elementwise
  work slows every DVE op ~20% and its strict FIFO head-of-line blocks
  cross-engine chains. Keep gpsimd idle.
- Each dma_start costs ~0.6us of serialized SP sequencer time (DIRECT2D),
  and consumers wait on whole-DMA semaphores: use FEW transfers, sized to
  need-granularity, issued in consumption order. The keep mask is
  host-swizzled to [p, sb, c, s'] so one s-block is one DMA with
  16KB-contiguous runs per partition.
- Software pipeline DEPTH=2 (deeper measured slower); the final block
  drains with the back phase split into s-halves to shorten the tail.
Host side only slices/transposes/casts inputs (layout prep).
"""
import sys, math
import numpy as np

sys.path.insert(0, "/opt/trn_rl_repo")

SQ, B, NH, HN = 2048, 2, 16, 128
NCORES = 8
HPC = 4                      # heads per core
TC = SQ // 128               # 16 t-chunks
SBLK = 512                   # s-block width
NSB = SQ // SBLK             # 4 s-blocks
SCALE = 1.0 / math.sqrt(128.0)   # COEFF / NORM_FACTOR = 1/sqrt(hn)
KDVE = 12                        # t-chunks whose column-sums go via DVE adds
DEPTH = 2                        # software pipeline depth (F blocks ahead of B)

_CACHE = {}


def _build(repeat=1):
    import concourse.bacc as bacc
    import concourse.tile as tile
    from concourse import mybir

    F32, BF16, U32 = mybir.dt.float32, mybir.dt.bfloat16, mybir.dt.uint32
    EXP = mybir.ActivationFunctionType.Exp
    AND = mybir.AluOpType.logical_and

    nc = bacc.Bacc(None, target_bir_lowering=False)
    qT_d = nc.dram_tensor("qT", [HPC, HN, SQ], BF16, kind="ExternalInput")
    kT_d = nc.dram_tensor("kT", [HPC, HN, SQ], BF16, kind="ExternalInput")
    v_d = nc.dram_tensor("v", [HPC, SQ, HN], BF16, kind="ExternalInput")
    # keep swizzled host-side to [p, sb, c, s'] so one s-block is a single
    # DMA with 16KB-contiguous runs per partition (descriptor-efficient)
    keep_d = nc.dram_tensor("keepT", [128, NSB, TC, SBLK], BF16,
                            kind="ExternalInput")
    ctxT_d = nc.dram_tensor("ctxT", [HPC, HN, SQ], BF16, kind="ExternalOutput")

    with tile.TileContext(nc) as tc:
        with (
            tc.tile_pool(name="sbkeep", bufs=1) as sbkeep,
            tc.tile_pool(name="const", bufs=1) as const,
            tc.tile_pool(name="sbqkv", bufs=3) as sbqkv,
            tc.tile_pool(name="sbpt", bufs=DEPTH + 1) as sbpt,
            tc.tile_pool(name="sbacc", bufs=DEPTH + 1) as sbacc,
            tc.tile_pool(name="sbtmp", bufs=2) as sbtmp,
            tc.tile_pool(name="sbe", bufs=4) as sbe,
            tc.tile_pool(name="sbmisc", bufs=3) as sbmisc,
            tc.tile_pool(name="pst", bufs=2, space="PSUM") as pst,
            tc.tile_pool(name="psc", bufs=2, space="PSUM") as psc,
            tc.tile_pool(name="pss", bufs=2, space="PSUM") as pss,
        ):
            keep_t = sbkeep.tile([128, NSB, TC, SBLK], BF16, tag="keep")

            ones_b = const.tile([128, 1], BF16, tag="ob")
            nc.vector.memset(ones_b[:], 1.0)
            # full ones stationary: the sums joins then produce the column
            # sums replicated across all 128 partitions (same N=512
            # streaming cost as an M=1 join), so no partition_broadcast
            # is ever needed for the normalize
            ones_m = const.tile([128, 128], BF16, tag="om")
            nc.vector.memset(ones_m[:], 1.0)
            warm_src = const.tile([128, SBLK], BF16, tag="warm")
            nc.vector.memset(warm_src[:], 0.0)
            warm_e = const.tile([128, 16], BF16, tag="warme")

            def emit_front(h, sb, qT_t, kT_t):
                """scores -> exp -> mask for (h, sb); returns (pt, acc).

                Mask is a u32 bitwise AND (keep pattern 0xFFFF/0x0000);
                the q=3 group's AND runs on GPSIMD (its chunks feed only
                PE matmuls, so no DVE chain depends on it). Chunks
                0..KDVE-1 are pair-tree-reduced on the DVE into
                acc[128, 2, SBLK]; the PE later contracts chunks
                KDVE..15 plus the two acc rows.
                """
                s0 = sb * SBLK
                pt = sbpt.tile([128, TC, SBLK], BF16, tag="pt")
                acc = sbacc.tile([128, 2, SBLK], BF16, tag="acc")
                tmp = sbtmp.tile([128, 2, SBLK], BF16, tag="tmp")
                # NOTE: gpsimd must not take any of this work — it shares
                # an SBUF port with the DVE, and heavy gpsimd elementwise
                # traffic slows every DVE op by ~20% (measured)
                for h8 in range(2):
                    e16 = sbe.tile([128, 8, SBLK], BF16, tag="e")
                    for g in range(4):
                        st = pst.tile([128, 2, SBLK], F32, tag="st")
                        for j in range(2):
                            ti = 8 * h8 + 2 * g + j
                            nc.tensor.matmul(
                                st[:, j, :],
                                kT_t[:, 128 * ti:128 * (ti + 1)],
                                qT_t[:, s0:s0 + SBLK],
                                start=True, stop=True)
                        nc.scalar.activation(
                            e16[:, 2 * g:2 * g + 2, :], st[:], EXP,
                            scale=SCALE)
                    c0 = 8 * h8
                    nc.vector.tensor_mul(
                        pt[:, c0:c0 + 8, :], e16[:],
                        keep_t[:, sb, c0:c0 + 8, :])
                    if h8 == 0:
                        nc.vector.tensor_add(acc[:], pt[:, 0:2, :],
                                             pt[:, 2:4, :])
                        nc.vector.tensor_add(tmp[:], pt[:, 4:6, :],
                                             pt[:, 6:8, :])
                        nc.vector.tensor_add(acc[:], acc[:], tmp[:])
                    else:
                        nc.vector.tensor_add(tmp[:], pt[:, 8:10, :],
                                             pt[:, 10:12, :])
                        nc.vector.tensor_add(acc[:], acc[:], tmp[:])
                return pt, acc

            def emit_back(h, sb, pt, acc, v_t, nsplit=1, shalves=1):
                """sums -> PV -> normalize -> store for (h, sb).

                shalves > 1 splits the whole back phase into s-column
                halves so the drain chain after the final PV matmul is
                half as long (used for the last pipeline block only).
                """
                s0 = sb * SBLK
                sw = SBLK // shalves
                for so in range(0, SBLK, sw):
                    sums_p = pss.tile([128, SBLK], F32, tag="sums")
                    for ti in range(KDVE, TC):
                        nc.tensor.matmul(sums_p[:, :sw], ones_m[:],
                                         pt[:, ti, so:so + sw],
                                         start=(ti == KDVE), stop=False)
                    nc.tensor.matmul(sums_p[:, :sw], ones_m[:],
                                     acc[:, 0, so:so + sw],
                                     start=False, stop=False)
                    nc.tensor.matmul(sums_p[:, :sw], ones_m[:],
                                     acc[:, 1, so:so + sw],
                                     start=False, stop=True)
                    ctx_p = psc.tile([128, SBLK], F32, tag="ctx")
                    for ti in range(TC):
                        nc.tensor.matmul(ctx_p[:, :sw], v_t[:, ti, :],
                                         pt[:, ti, so:so + sw],
                                         start=(ti == 0), stop=(ti == TC - 1))
                    w = sw // nsplit
                    for o in range(0, sw, w):
                        recip = sbmisc.tile([128, w], F32, tag="recip")
                        nc.vector.reciprocal_approx_fast(
                            recip[:], sums_p[:, o:o + w])
                        ctx_s = sbmisc.tile([128, w], BF16, tag="ctxs")
                        nc.vector.tensor_mul(ctx_s[:], ctx_p[:, o:o + w],
                                             recip[:])
                        nc.sync.dma_start(
                            out=ctxT_d[h, :, s0 + so + o:s0 + so + o + w],
                            in_=ctx_s[:])

            def body(_iv=None):
                # warm the PE clock (HAM) and the ACT exp table with dummy
                # ops that only depend on the memset, while the first DMAs
                # land
                warm_p = pss.tile([1, SBLK], F32, tag="sums")
                for _ in range(6):
                    nc.tensor.matmul(warm_p[:], ones_b[:], warm_src[:],
                                     start=True, stop=True)
                nc.scalar.activation(warm_e[:], warm_src[:, 0:16], EXP,
                                     scale=SCALE)

                pendings = []   # [(h, sb, pt, acc, v_t), ...]
                qkv = {}
                for h in range(HPC):
                    qT_t = sbqkv.tile([128, SQ], BF16, tag="qT")
                    kT_t = sbqkv.tile([128, SQ], BF16, tag="kT")
                    v_t = sbqkv.tile([128, TC, HN], BF16, tag="v")
                    v_r = v_d[h].rearrange("(c p) d -> p c d", p=128)
                    if h == 0:
                        # need-ordered loads, trigger-frugal: each
                        # dma_start costs ~0.6us of SP sequencer time
                        # (DIRECT2D), and consumers wait on whole-DMA
                        # semaphores — so the first-needed operands go
                        # as small pieces and the bulk as few large
                        # descriptor-efficient transfers.
                        # qT s-block 0 + first kT chunks gate compute
                        # start; issue them from the SECOND HWDGE engine
                        # (ACT, idle until its first exp at ~12us) so
                        # their DIRECT2D triggers fire in parallel with
                        # the SP's stream below
                        for c in range(2):
                            nc.scalar.dma_start(
                                out=qT_t[:, 256 * c:256 * (c + 1)],
                                in_=qT_d[h][:, 256 * c:256 * (c + 1)])
                        # kT as 16 x 128-col pieces, alternating between
                        # the two HWDGE trigger engines for ring
                        # parallelism without delaying SP's keep triggers
                        for c in range(16):
                            eng = nc.scalar if c % 2 else nc.sync
                            eng.dma_start(
                                out=kT_t[:, 128 * c:128 * (c + 1)],
                                in_=kT_d[h][:, 128 * c:128 * (c + 1)])
                        for qtr in range(4):
                            nc.sync.dma_start(
                                out=keep_t[:, 0, 4 * qtr:4 * (qtr + 1), :],
                                in_=keep_d[:, 0, 4 * qtr:4 * (qtr + 1), :])
                        nc.sync.dma_start(out=qT_t[:, SBLK:2 * SBLK],
                                          in_=qT_d[h][:, SBLK:2 * SBLK])
                        for half in range(2):
                            nc.sync.dma_start(
                                out=keep_t[:, 1, 8 * half:8 * (half + 1), :],
                                in_=keep_d[:, 1, 8 * half:8 * (half + 1), :])
                        for c in range(2, NSB):
                            nc.sync.dma_start(
                                out=qT_t[:, SBLK * c:SBLK * (c + 1)],
                                in_=qT_d[h][:, SBLK * c:SBLK * (c + 1)])
                        for half in range(2):
                            nc.sync.dma_start(
                                out=v_t[:, 8 * half:8 * (half + 1), :],
                                in_=v_r[:, 8 * half:8 * (half + 1), :])
                        for half in range(2):
                            nc.sync.dma_start(
                                out=keep_t[:, 2, 8 * half:8 * (half + 1), :],
                                in_=keep_d[:, 2, 8 * half:8 * (half + 1), :])
                        nc.sync.dma_start(out=keep_t[:, 3], in_=keep_d[:, 3])
                    else:
                        for half in range(2):
                            cols = slice(SQ // 2 * half, SQ // 2 * (half + 1))
                            nc.sync.dma_start(out=qT_t[:, cols],
                                              in_=qT_d[h][:, cols])
                            nc.sync.dma_start(out=kT_t[:, cols],
                                              in_=kT_d[h][:, cols])
                            nc.sync.dma_start(
                                out=v_t[:, 8 * half:8 * (half + 1), :],
                                in_=v_r[:, 8 * half:8 * (half + 1), :])
                    qkv[h] = (qT_t, kT_t, v_t)
                    last_head = h == HPC - 1
                    for sb in range(NSB):
                        # shallow out the pipeline before the final block
                        # so the drain tail after the last front is short
                        if last_head and sb == NSB - 1:
                            while len(pendings) > 1:
                                emit_back(*pendings.pop(0))
                        pt, acc = emit_front(h, sb, qT_t, kT_t)
                        pendings.append((h, sb, pt, acc, v_t))
                        if len(pendings) > DEPTH:
                            emit_back(*pendings.pop(0))
                for i, p in enumerate(pendings):
                    emit_back(*p, shalves=1 + i)

            if repeat == 1:
                body()
            else:
                with tc.For_i(0, repeat, 1):
                    body()
    nc.compile()
    return nc


def _get_nc(repeat=1):
    if repeat not in _CACHE:
        _CACHE[repeat] = _build(repeat)
    return _CACHE[repeat]


def _make_in_maps(query_layer, key_layer, value_layer, attention_mask):
    import ml_dtypes
    bf16 = ml_dtypes.bfloat16
    q = np.asarray(query_layer, dtype=np.float32)
    k = np.asarray(key_layer, dtype=np.float32)
    v = np.asarray(value_layer, dtype=np.float32)
    m = np.asarray(attention_mask)
    in_maps = []
    for c in range(NCORES):
        b = c // 4
        h0 = 4 * (c % 4)
        hs = slice(h0, h0 + HPC)
        qT = np.ascontiguousarray(
            q[:, b, hs, :].transpose(1, 2, 0)).astype(bf16)    # [4,hn,sq]
        kT = np.ascontiguousarray(
            k[:, b, hs, :].transpose(1, 2, 0)).astype(bf16)
        vv = np.ascontiguousarray(
            v[:, b, hs, :].transpose(1, 0, 2)).astype(bf16)    # [4,sq,hn]
        # keep swizzled to [p, sb, c, s']: keep2[p, sb, c, s'] =
        # keep[t = c*128 + p, s = sb*512 + s']
        keep_ts = (m[b, 0] == 0).T.astype(bf16)                # [t,s] bf16
        keepT = np.ascontiguousarray(
            keep_ts.reshape(16, 128, 4, 512).transpose(1, 2, 0, 3))
        in_maps.append({"qT": qT, "kT": kT, "v": vv, "keepT": keepT})
    return in_maps


def _run(nc, in_maps):
    from concourse.bass_utils import run_bass_kernel_spmd
    return run_bass_kernel_spmd(nc, in_maps, list(range(NCORES)))


def kernel(query_layer, key_layer, value_layer, attention_mask):
    in_maps = _make_in_maps(query_layer, key_layer, value_layer, attention_mask)
    res = _run(_get_nc(1), in_maps)
    out = np.empty((SQ, B, NH, HN), dtype=np.float32)
    for c in range(NCORES):
        b = c // 4
        h0 = 4 * (c % 4)
        ctxT = np.asarray(res.results[c]["ctxT"], dtype=np.float32)   # [4,hn,sq]
        out[:, b, h0:h0 + HPC, :] = ctxT.transpose(2, 0, 1)
    return out.reshape(SQ, B, NH * HN)
